# revision 20
# baseline (speedup 1.0000x reference)
"""Trainium2 Bass kernel for nn_BasicCNN (conv bank + LoRA-masked recurrent net).

Pure data-parallel over batch (128 rows/core on 8 cores), ZERO collectives —
under the conservative collective cost model (15us + bytes/40GBps) any
TP-style per-timestep AllGather dwarfs the actual compute, so each core runs
the full network on its batch shard instead:

 - W1 = W + 2*(A@B)*mask + I is built on HOST (fp32) and shipped bf16.
   The +I fold implements the residual; rows 0:KT_RES*128 stay SBUF-resident,
   the rest is streamed per timestep as [128, 1024] column-chunks (one chunk
   per (m-group, k-slab)) on the SP/Pool DMA queues, double-buffered through
   an 8-slot ring so DMA hides under the PE matmuls.
 - State kept transposed k-major ([state_dim, batch] in 32 slabs of
   [128, 128]) so W tiles are the stationary operand and the matmul output
   [m-part, batch] is directly the next state slab — no transposes ever.
 - PSUM allows only 8 bank-aligned accumulators, so each timestep runs 4
   m-groups x 8 banks; relu drains alternate DVE/Act so banks free fast.
 - t1 contracts only the sensory block (state1 is zero past SEN);
   t4 computes only the O block; conv bank = one dense [512, 3328] matmul
   vs host-assembled scatter of the conv kernels; out projection streamed.
"""
import sys

for _p in ("/opt/trn_rl_repo", "/root/.axon_site/_ro/trn_rl_repo"):
    if _p not in sys.path:
        sys.path.append(_p)

import numpy as np
import ml_dtypes

import concourse.bacc as bacc
import concourse.mybir as mybir
import concourse.tile as tile
from concourse.bass_utils import run_bass_kernel_spmd

dt = mybir.dt
BF16 = ml_dtypes.bfloat16
AF = mybir.ActivationFunctionType

N_CORES = 8
B = 1024
HW = 8
C_IN = 8
FN = 16
SEN, INT, OUT = 1024, 2048, 1024
TOT = 4096
CNN_OUT = 3264
CNN_PAD = 3328
NUM_OUT = 1968
NUM_PAD = 2048
LORA_SCALE = 2.0

BSH = B // N_CORES           # 128 batch rows per core
KT = TOT // 128              # 32 k-slabs of state/W
KT_RES = 12                  # W1 k-slabs resident in SBUF
KT_STR = KT - KT_RES         # 19 streamed per full timestep
NG = 4                       # m-groups per timestep
GM = 8                       # m-tiles per group (= PSUM banks)
CONV_MT = CNN_PAD // 128     # 26
SEN_KT = SEN // 128          # 8
OUT_KT = OUT // 128          # 8 (O-block k-slabs for out proj)
OMT = NUM_PAD // 128         # 16


def _build_program(reps: int = 1, use_cc: bool = True):
    nc = bacc.Bacc("TRN2", target_bir_lowering=False, debug=False,
                   enable_asserts=True, num_devices=N_CORES)

    xT_d = nc.dram_tensor("xT", [512, BSH], dt.bfloat16, kind="ExternalInput")
    wbig_d = nc.dram_tensor("wbig", [512, CNN_PAD], dt.bfloat16, kind="ExternalInput")
    cbias_d = nc.dram_tensor("cbias", [CNN_PAD], dt.float32, kind="ExternalInput")
    ipw_d = nc.dram_tensor("ipw", [CNN_PAD, SEN], dt.bfloat16, kind="ExternalInput")
    ipb_d = nc.dram_tensor("ipb", [SEN], dt.float32, kind="ExternalInput")
    w1r_d = nc.dram_tensor("w1r", [KT_RES * 128, TOT], dt.bfloat16, kind="ExternalInput")
    w1s_d = nc.dram_tensor("w1s", [KT_STR * 128, TOT], dt.bfloat16, kind="ExternalInput")
    oww_d = nc.dram_tensor("oww", [OUT, NUM_PAD], dt.bfloat16, kind="ExternalInput")
    ob_d = nc.dram_tensor("ob", [NUM_PAD], dt.float32, kind="ExternalInput")
    obb_d = nc.dram_tensor("obb", [1, NUM_PAD], dt.bfloat16, kind="ExternalInput")

    # partition-major output layout [p, m, b]; host maps row j = m*128+p
    outT_d = nc.dram_tensor("outT", [128, OMT, BSH], dt.float32, kind="ExternalOutput")

    with tile.TileContext(nc) as tc:
        with tc.tile_pool(name="pers", bufs=1) as pers, \
             tc.tile_pool(name="psum", bufs=8, space="PSUM") as psp, \
             tc.tile_pool(name="wbigp", bufs=4) as wbp, \
             tc.tile_pool(name="ipwp", bufs=8) as ipp, \
             tc.tile_pool(name="wchk", bufs=11) as wcp, \
             tc.tile_pool(name="owwp", bufs=4) as owp, \
             tc.tile_pool(name="outp", bufs=1) as otp:

            wres = pers.tile([128, KT_RES, TOT], dt.bfloat16, tag="wres")
            st_a = pers.tile([128, KT, BSH], dt.bfloat16, tag="st_a")
            st_b = pers.tile([128, KT, BSH], dt.bfloat16, tag="st_b")
            ostate = pers.tile([128, OUT_KT, BSH], dt.bfloat16, tag="ostate")
            featT = pers.tile([128, CONV_MT, BSH], dt.bfloat16, tag="featT")
            xT_sb = pers.tile([128, 4, BSH], dt.bfloat16, tag="xT_sb")
            cbias_sb = pers.tile([128, CONV_MT], dt.float32, tag="cbias_sb")
            ipb_sb = pers.tile([128, SEN_KT], dt.float32, tag="ipb_sb")
            ob_sb = pers.tile([128, OMT], dt.float32, tag="ob_sb")

            # unified round-robin over the three DMA-capable queues; all
            # elementwise drains live on DVE so no queue mixes DMA + compute
            dmaq = [nc.gpsimd, nc.sync, nc.scalar]
            qctr = [0]

            def dma(out, in_):
                dmaq[qctr[0] % 3].dma_start(out=out, in_=in_)
                qctr[0] += 1

            warm_in = pers.tile([128, 1], dt.float32, tag="warm_in")
            warm_out = pers.tile([128, 1], dt.float32, tag="warm_out")
            ones_sb = pers.tile([1, BSH], dt.bfloat16, tag="ones_sb")
            obb_sb = pers.tile([1, NUM_PAD], dt.bfloat16, tag="obb_sb")
            nc.gpsimd.memset(warm_in[:], 0.0)
            nc.gpsimd.memset(ones_sb[:], 1.0)
            nc.sync.dma_start(out=obb_sb[:], in_=obb_d[:, :])

            nc.gpsimd.dma_start(out=xT_sb[:, :, :],
                                in_=xT_d.rearrange("(k p) b -> p k b", p=128))
            nc.sync.dma_start(out=cbias_sb[:], in_=cbias_d.rearrange("(m p) -> p m", p=128))
            nc.sync.dma_start(out=ipb_sb[:], in_=ipb_d.rearrange("(m p) -> p m", p=128))
            nc.sync.dma_start(out=ob_sb[:], in_=ob_d.rearrange("(m p) -> p m", p=128))

            for rep in range(reps):
                # ---- conv bank: one dense matmul vs scattered conv kernels ----
                # wbig loaded as 16 column-chunks round-robin over all three
                # DMA queues so conv's first m-tiles start ~2.5us in
                wbig_t = [wbp.tile([128, CNN_PAD], dt.bfloat16, tag="wbig",
                                   name=f"wbig{rep}_{i}") for i in range(4)]
                for c in range(4):
                    c0, c1 = c * 832, min((c + 1) * 832, CNN_PAD)
                    for kk in range(4):
                        dma(wbig_t[kk][:, c0:c1], wbig_d[kk * 128:(kk + 1) * 128, c0:c1])
                for m in range(CONV_MT):
                    ps = psp.tile([128, BSH], dt.float32, tag="ps", name=f"cv{rep}_{m}")
                    for kk in range(4):
                        nc.tensor.matmul(ps[:], wbig_t[kk][:, m * 128:(m + 1) * 128],
                                         xT_sb[:, kk, :], start=(kk == 0), stop=(kk == 3))
                    nc.vector.tensor_scalar(featT[:, m, :], ps[:], cbias_sb[:, m:m + 1],
                                            0.0, op0=mybir.AluOpType.add,
                                            op1=mybir.AluOpType.max)

                # ---- input proj -> state1 (k-outer over 26 ipw slabs) ----
                ip_ps = [psp.tile([128, BSH], dt.float32, tag="ps", name=f"ip{rep}_{m}")
                         for m in range(SEN_KT)]
                for k in range(CONV_MT):
                    t = ipp.tile([128, SEN], dt.bfloat16, tag="ipw")
                    dma(t[:], ipw_d[k * 128:(k + 1) * 128, :])
                    for m in range(SEN_KT):
                        nc.tensor.matmul(ip_ps[m][:], t[:, m * 128:(m + 1) * 128],
                                         featT[:, k, :], start=(k == 0),
                                         stop=(k == CONV_MT - 1))
                for m in range(SEN_KT):
                    nc.vector.tensor_scalar(st_a[:, m, :], ip_ps[m][:],
                                            ipb_sb[:, m:m + 1], 0.0,
                                            op0=mybir.AluOpType.add,
                                            op1=mybir.AluOpType.max)

                # ---- t1 (contracts only the SEN block, k0..8): streamed as
                # [128, 1024] chunks in exact consumption order so PE never
                # waits on a coarse resident slab at startup ----
                cur, nxt = st_a, st_b
                for mg in range(NG):
                    ps = [psp.tile([128, BSH], dt.float32, tag="ps",
                                   name=f"t1r{rep}g{mg}_{i}") for i in range(GM)]
                    for k in range(SEN_KT):
                        chunk = wcp.tile([128, 1024], dt.bfloat16, tag="wchk")
                        dma(chunk[:], w1r_d[k * 128:(k + 1) * 128,
                                            mg * 1024:(mg + 1) * 1024])
                        for m8 in range(GM):
                            nc.tensor.matmul(ps[m8][:],
                                             chunk[:, m8 * 128:(m8 + 1) * 128],
                                             cur[:, k, :],
                                             start=(k == 0), stop=(k == SEN_KT - 1))
                    for m8 in range(GM):
                        nc.vector.tensor_scalar_max(nxt[:, mg * GM + m8, :],
                                                    ps[m8][:], 0.0)
                    if rep == 0:
                        # resident W1 slabs in [128, 1024] pieces, interleaved
                        # behind each t1 group so t2's low-k slabs land first
                        # without head-of-line-blocking t1's own stream
                        for kr in range(3 * mg, min(3 * mg + 3, KT_RES)):
                            for mg2 in range(NG):
                                dma(wres[:, kr, mg2 * 1024:(mg2 + 1) * 1024],
                                    w1r_d[kr * 128:(kr + 1) * 128,
                                          mg2 * 1024:(mg2 + 1) * 1024])
                cur, nxt = nxt, cur

                # warm the Act engine's activation table (Copy) off the
                # critical path so the tail bias-adds pay no table load
                if rep == 0:
                    nc.scalar.activation(warm_out[:], warm_in[:], AF.Copy, bias=0.0)

                # ---- recurrence t2..t3 (full contraction) ----
                for t in (2, 3):
                    for mg in range(NG):
                        ps = [psp.tile([128, BSH], dt.float32, tag="ps",
                                       name=f"t{t}r{rep}g{mg}_{i}") for i in range(GM)]
                        chunk = None
                        for k in range(KT):
                            if k >= KT_RES:
                                chunk = wcp.tile([128, 1024], dt.bfloat16, tag="wchk")
                                dma(chunk[:],
                                    w1s_d[(k - KT_RES) * 128:(k - KT_RES + 1) * 128,
                                          mg * 1024:(mg + 1) * 1024])
                            for m8 in range(GM):
                                m = mg * GM + m8
                                if k < KT_RES:
                                    wap = wres[:, k, m * 128:(m + 1) * 128]
                                else:
                                    wap = chunk[:, m8 * 128:(m8 + 1) * 128]
                                nc.tensor.matmul(ps[m8][:], wap, cur[:, k, :],
                                                 start=(k == 0), stop=(k == KT - 1))
                        for m8 in range(GM):
                            nc.vector.tensor_scalar_max(nxt[:, mg * GM + m8, :],
                                                        ps[m8][:], 0.0)
                    cur, nxt = nxt, cur

                # prefetch out-proj weights for jg0 k0..3 ahead of t4 so the
                # t4 -> outproj boundary has no weight-arrival bubble
                oww_pre = [owp.tile([128, 1024], dt.bfloat16, tag="oww",
                                    name=f"owwpre{rep}_{k}") for k in range(4)]
                for k in range(4):
                    dma(oww_pre[k][:], oww_d[k * 128:(k + 1) * 128, 0:1024])

                # ---- t4: only the O block (m-group 3) ----
                t4_ps = [psp.tile([128, BSH], dt.float32, tag="ps",
                                  name=f"t4r{rep}_{i}") for i in range(GM)]
                chunk = None
                for k in range(KT):
                    if k >= KT_RES:
                        chunk = wcp.tile([128, 1024], dt.bfloat16, tag="wchk")
                        dma(chunk[:], w1s_d[(k - KT_RES) * 128:(k - KT_RES + 1) * 128,
                                            3 * 1024:4 * 1024])
                    for m8 in range(GM):
                        m = 3 * GM + m8
                        if k < KT_RES:
                            wap = wres[:, k, m * 128:(m + 1) * 128]
                        else:
                            wap = chunk[:, m8 * 128:(m8 + 1) * 128]
                        nc.tensor.matmul(t4_ps[m8][:], wap, cur[:, k, :],
                                         start=(k == 0), stop=(k == KT - 1))
                for m8 in range(GM):
                    nc.vector.tensor_scalar_max(ostate[:, m8, :], t4_ps[m8][:], 0.0)

                # ---- output projection (2 groups x 8 banks, oww streamed) ----
                ostg = otp.tile([128, OMT, BSH], dt.float32, tag="ostg")
                for jg in range(2):
                    op_ps = [psp.tile([128, BSH], dt.float32, tag="ps",
                                      name=f"op{rep}g{jg}_{i}") for i in range(GM)]
                    for k in range(OUT_KT):
                        if jg == 0 and k < 4:
                            ch = oww_pre[k]
                        else:
                            ch = owp.tile([128, 1024], dt.bfloat16, tag="oww")
                            dma(ch[:], oww_d[k * 128:(k + 1) * 128,
                                             jg * 1024:(jg + 1) * 1024])
                        for m8 in range(GM):
                            nc.tensor.matmul(op_ps[m8][:], ch[:, m8 * 128:(m8 + 1) * 128],
                                             ostate[:, k, :], start=(k == 0),
                                             stop=False)
                    # bias folded in as a ones-row matmul closing each
                    # accumulation group; drains are then pure copies split
                    # across DVE and Act, stored in 4-column quarters
                    for m8 in range(GM):
                        jm = jg * GM + m8
                        nc.tensor.matmul(op_ps[m8][:],
                                         obb_sb[:, jm * 128:(jm + 1) * 128],
                                         ones_sb[:, :], start=False, stop=True)
                    for half in range(2):
                        for m8 in range(4 * half, 4 * half + 4):
                            jm = jg * GM + m8
                            if m8 % 2 == 0:
                                nc.vector.tensor_scalar_add(ostg[:, jm, :],
                                                            op_ps[m8][:], 0.0)
                            else:
                                nc.scalar.activation(ostg[:, jm, :], op_ps[m8][:],
                                                     AF.Copy, bias=0.0)
                        lo = jg * GM + 4 * half
                        dma(outT_d[:, lo:lo + 4, :], ostg[:, lo:lo + 4, :])

    nc.compile()
    return nc


_PROGRAM_CACHE: dict = {}


def get_program(reps: int = 1, use_cc: bool = True):
    key = (reps, use_cc)
    if key not in _PROGRAM_CACHE:
        _PROGRAM_CACHE[key] = _build_program(reps, use_cc)
    return _PROGRAM_CACHE[key]


def _assemble_wbig(inputs):
    wbig = np.zeros((512, CNN_PAD), np.float32)
    cbias = np.zeros(CNN_PAD, np.float32)
    off = 0
    for k in range(1, 9):
        o = HW - k + 1
        w = np.asarray(inputs[f"conv_w{k}"], np.float32)
        cb = np.asarray(inputs["conv_b"], np.float32)[k - 1]
        py = np.arange(o)[:, None, None]
        px = np.arange(o)[None, :, None]
        cc = np.arange(C_IN)[None, None, :]
        ncol = np.arange(FN)[:, None, None]
        cols = off + ncol * o * o + py[None, :, :, 0] * o + px[None, :, :, 0]
        for dy in range(k):
            for dx in range(k):
                rows = (py + dy) * 64 + (px + dx) * 8 + cc
                wbig[rows[None, :, :, :], cols[:, :, :, None]] = \
                    w[:, :, dy, dx][:, None, None, :]
        cbias[off + np.arange(FN * o * o)] = np.repeat(cb, o * o)
        off += FN * o * o
    return wbig, cbias


def _prep_inputs(inputs):
    x = np.asarray(inputs["x"], np.float32)
    W = np.asarray(inputs["W"], np.float32)
    lora_A = np.asarray(inputs["lora_A"], np.float32)
    lora_B = np.asarray(inputs["lora_B"], np.float32)
    ip_w = np.asarray(inputs["ip_w"], np.float32)
    ip_b = np.asarray(inputs["ip_b"], np.float32)
    out_w = np.asarray(inputs["out_w"], np.float32)
    out_b = np.asarray(inputs["out_b"], np.float32)

    wbig, cbias = _assemble_wbig(inputs)
    ipw_pad = np.zeros((CNN_PAD, SEN), np.float32)
    ipw_pad[:CNN_OUT] = ip_w
    oww_pad = np.zeros((OUT, NUM_PAD), np.float32)
    oww_pad[:, :NUM_OUT] = out_w
    ob_pad = np.zeros(NUM_PAD, np.float32)
    ob_pad[:NUM_OUT] = out_b

    mask = (W != 0).astype(np.float32)
    W1 = W + LORA_SCALE * (lora_A @ lora_B) * mask + np.eye(TOT, dtype=np.float32)

    def bf(a):
        return np.ascontiguousarray(a).astype(BF16)

    shared = {
        "wbig": bf(wbig), "cbias": np.ascontiguousarray(cbias),
        "ipw": bf(ipw_pad), "ipb": np.ascontiguousarray(ip_b),
        "w1r": bf(W1[:KT_RES * 128]), "w1s": bf(W1[KT_RES * 128:]),
        "oww": bf(oww_pad), "ob": np.ascontiguousarray(ob_pad),
        "obb": bf(ob_pad.reshape(1, NUM_PAD)),
    }
    in_maps = []
    for c in range(N_CORES):
        m = dict(shared)
        m["xT"] = bf(x[c * BSH:(c + 1) * BSH].reshape(BSH, 512).T)
        in_maps.append(m)
    return in_maps


def run_on_hw(in_maps, reps: int = 1):
    nc = get_program(reps)
    return run_bass_kernel_spmd(nc, in_maps, list(range(N_CORES)), trace=False)


def kernel(**inputs) -> np.ndarray:
    in_maps = _prep_inputs(inputs)
    res = run_on_hw(in_maps, reps=1)
    out = np.zeros((B, NUM_OUT), np.float32)
    for c in range(N_CORES):
        o = np.asarray(res.results[c]["outT"], np.float32)  # [128, OMT, BSH]
        o = o.transpose(1, 0, 2).reshape(NUM_PAD, BSH)      # row j = m*128+p
        out[c * BSH:(c + 1) * BSH, :] = o[:NUM_OUT].T
    return np.ascontiguousarray(out)
